# revision 28
# baseline (speedup 1.0000x reference)
"""BNext block (attention + FFN_1x1, binarized convs, frozen BN) on 8 TRN2 cores.

Data-parallel over batch (16 -> 2 images per core). Per core:
  - channels on partitions (2 c-tiles of 128), pixels (b, h, w) on the free dim
  - hardsign z computed by ScalarE Sign into a zero-padded fp8e4 buffer holding
    BOTH c-tile planes; 3x3 binary conv = 9 shifted fp8 DoubleRow matmuls per
    (j-chunk, mtile) contracting 256 channels each, accumulated in PSUM;
    weights are pre-scaled by 2^13 on the host so they sit in e4m3's normal
    range (descale folded into the bn1 scale applied at the PSUM drain)
  - PSUM tiles hold 2 j-chunks (2 banks); bn1+prelu drains handle a pair per
    ScalarE Prelu instruction with accum_out producing SE-pool sums free
  - SE means are linear: mean(mix) = s*mean(x) + (1-s)*mean(y); the s/(1-s)
    factors are folded into the SE w1 weights on the host so the pooled sums
    feed the 1x1 SE matmuls directly
  - residual/bn2 algebra collapses to outa = residual*(inv2*gate*y1 + 1)
    (bias2 folded into the z2 threshold / final bias / sum corrections)
  - elementwise work is spread across engines: ScalarE keeps Sign + the PSUM
    Prelu drains, VectorE does the gate/residual multiplies in bf16 (2x/4x DVE
    modes where the ops support them); inv2/finv2 > 0 are folded through the
    prelu into the bn1 drain affine so the SE gates are raw sigmoids
  - DMA rings are split so loop iterations overlap: x loads + consts on the
    SP HWDGE ring, bf16->fp32 casting output stores on the Pool SWDGE ring
  - per-image pipelining within an iteration, plus ping-pong double buffering
    of all per-iteration state (including PSUM pools split conv-vs-ffn) so
    consecutive bench-loop iterations overlap (the steady-state bench measures
    throughput = max per-engine busy time)
"""

import numpy as np

EPS = 1e-5
NCORES = 8
B, C, H, W = 16, 256, 56, 56
BP = B // NCORES            # images per core
HW = H * W                  # 3136
PIX = BP * HW               # 6272
CT = C // 128               # 2 c-tiles
HP, WP = H + 2, W + 2       # padded z: 58 x 58
ZPL = BP * HP * WP          # fp8 z plane length (6728)
RS = 8                      # conv chunk rows  -> N = 448
NCH = H // RS               # 7 conv chunks per image
NPAIR = 4                   # drain pairs per image: (0,1),(2,3),(4,5),(6,)
SR = 28                     # A1 stream chunk rows (1568 elems)
NSC = H // SR               # 2 per image
SR2 = 28                    # A4/F3 chunk rows (1568 elems)
NSC2 = H // SR2             # 2 per image
NV = 20
WSCL = 2.0 ** 13            # fp8 weight pre-scale (host)

_CACHE = {}


def _build_program(loop_R=None, phase_limit=99, bench_mode=False):
    import concourse.bass as bass
    import concourse.bacc as bacc
    import concourse.tile as tile
    from concourse import mybir

    AF = mybir.ActivationFunctionType
    ALU = mybir.AluOpType
    PM = mybir.MatmulPerfMode
    F32 = mybir.dt.float32
    F8 = mybir.dt.float8e4
    BF16 = mybir.dt.bfloat16

    nc = bacc.Bacc("TRN2", target_bir_lowering=False, debug=False)

    KIN = "Internal" if bench_mode else "ExternalInput"
    KOUT = "Internal" if bench_mode else "ExternalOutput"
    xin = nc.dram_tensor("xin", [BP, C, H, W], F32, kind=KIN).ap()
    wa = nc.dram_tensor("wa", [9, 128, CT, C], F8, kind=KIN).ap()
    wf = nc.dram_tensor("wf", [128, CT, C], F8, kind=KIN).ap()
    vecs_d = nc.dram_tensor("vecs", [CT, 128, NV], F32, kind=KIN).ap()
    b1a_d = nc.dram_tensor("b1a", [32, 1], F32, kind=KIN).ap()
    b1f_d = nc.dram_tensor("b1f", [32, 1], F32, kind=KIN).ap()
    # host folds s / (1-s) (and 1/HW) into the first SE matmul weights
    w1a_d = nc.dram_tensor("w1a", [2, CT, 128, 32], F32, kind=KIN).ap()
    w2a_d = nc.dram_tensor("w2a", [32, C], F32, kind=KIN).ap()
    w1f_d = nc.dram_tensor("w1f", [2, CT, 128, 32], F32, kind=KIN).ap()
    w2f_d = nc.dram_tensor("w2f", [32, C], F32, kind=KIN).ap()
    out_d = nc.dram_tensor("out", [BP, C, H, W], F32, kind=KOUT).ap()
    tick_d = (nc.dram_tensor("tick", [1, 8], F32, kind="ExternalOutput").ap()
              if bench_mode else None)

    x_v = xin.rearrange("b (ct p) h w -> ct p b (h w)", ct=CT)
    out_v = out_d.rearrange("b (ct p) h w -> ct p b (h w)", ct=CT)

    (V_MOVE, V_AL1, V_INV1, V_BIAS1, V_AL2, V_S, V_1MS, V_INV2, V_ZB2,
     V_FINV1, V_FBIAS1, V_FAL2, V_FS, V_1MFS, V_FINV2, V_CFIN, V_B2A,
     V_B2F, V_SB2HW, V_BIAS2) = range(NV)

    NSET = 2 if loop_R is not None else 1
    pairs = [(0, 1), (2, 3), (4, 5), (6,)]

    with tile.TileContext(nc) as tc:
        import contextlib
        es = contextlib.ExitStack()
        with es:
            consts = es.enter_context(tc.tile_pool(name="consts", bufs=1))
            big = es.enter_context(tc.tile_pool(name="big", bufs=1))
            stream = es.enter_context(tc.tile_pool(name="stream", bufs=2))
            psum = es.enter_context(tc.tile_pool(name="psum", bufs=2, space="PSUM"))
            psum_se = es.enter_context(
                tc.tile_pool(name="psum_se", bufs=1, space="PSUM"))

            # pin the activation table set once, before the loop
            scr = consts.tile([1, 2], F32, name="scr")
            nc.vector.memset(scr[:], 0.0)
            nc.scalar.activation(scr[:, 0:1], scr[:, 0:1], AF.Sigmoid,
                                 bias=0.0, scale=1.0)

            # ---- per-set state (ping-pong across loop iterations) ----
            def make_state(u):
                st = {}
                st["vecs"] = [consts.tile([128, NV], F32, name=f"vecs{ct}_{u}")
                              for ct in range(CT)]
                st["b1a"] = consts.tile([32, 1], F32, name=f"b1a_{u}")
                st["b1f"] = consts.tile([32, 1], F32, name=f"b1f_{u}")
                st["w1a"] = [[consts.tile([128, 32], F32, name=f"w1a{k}_{ct}_{u}")
                              for ct in range(CT)] for k in range(2)]
                st["w1f"] = [[consts.tile([128, 32], F32, name=f"w1f{k}_{ct}_{u}")
                              for ct in range(CT)] for k in range(2)]
                st["w2a"] = consts.tile([32, C], F32, name=f"w2a_{u}")
                st["w2f"] = consts.tile([32, C], F32, name=f"w2f_{u}")
                st["wconv"] = [consts.tile([128, CT, C], F8, name=f"wc{t}_{u}")
                               for t in range(9)]
                st["wffn"] = consts.tile([128, CT, C], F8, name=f"wfn_{u}")
                st["zpad"] = big.tile([128, CT, ZPL], F8, name=f"zpad_{u}",
                                      tag=f"zpad_{u}")
                st["y1"] = [[big.tile([128, HW], BF16, name=f"y1_{ct}_{b}_{u}",
                                      tag=f"s1_{ct}_{b}_{u}")
                             for b in range(BP)] for ct in range(CT)]
                st["outa"] = [big.tile([128, PIX], BF16, name=f"outa{ct}_{u}",
                                       tag=f"oa{ct}_{u}") for ct in range(CT)]
                st["sums2"] = [consts.tile([128, 16], F32, name=f"sums2_{ct}_{u}")
                               for ct in range(CT)]
                st["sxr"] = [consts.tile([128, 4], F32, name=f"sxr{ct}_{u}")
                             for ct in range(CT)]
                st["ps_y1"] = [consts.tile([128, 16], F32, name=f"ps_y1_{ct}_{u}")
                               for ct in range(CT)]
                st["ps_u"] = [consts.tile([128, 16], F32, name=f"ps_u_{ct}_{u}")
                              for ct in range(CT)]
                st["ps_x"] = [consts.tile([128, 4], F32, name=f"ps_x_{ct}_{u}")
                              for ct in range(CT)]
                st["ps_oa"] = [consts.tile([128, 4], F32, name=f"ps_oa_{ct}_{u}")
                               for ct in range(CT)]
                st["zp5"] = st["zpad"][:].rearrange(
                    "p k (b h w) -> p k b h w", h=HP, w=WP)
                st["uid"] = u
                return st

            states = [make_state(u) for u in range(NSET)]

            if loop_R is not None:
                assert loop_R % NSET == 0
                es.enter_context(tc.For_i(0, loop_R // NSET, 1))

            if bench_mode:
                tick_sb = consts.tile([1, 8], F32, name="tick_sb")
                nc.vector.memset(tick_sb[:], 1.0)
                nc.sync.dma_start(tick_d, tick_sb[:])

            # ===== helpers (all take the state dict) =====
            def load_consts(st):
                for ct in range(CT):
                    nc.sync.dma_start(st["vecs"][ct][:], vecs_d[ct])
                nc.sync.dma_start(st["b1a"][:], b1a_d)
                nc.sync.dma_start(st["b1f"][:], b1f_d)
                for k in range(2):
                    for ct in range(CT):
                        nc.sync.dma_start(st["w1a"][k][ct][:], w1a_d[k, ct])
                        nc.sync.dma_start(st["w1f"][k][ct][:], w1f_d[k, ct])
                nc.sync.dma_start(st["w2a"][:], w2a_d)
                nc.sync.dma_start(st["w2f"][:], w2f_d)
                for t in range(9):
                    nc.sync.dma_start(st["wconv"][t][:], wa[t])
                nc.sync.dma_start(st["wffn"][:], wf)
                # zpad borders (z2 writes corrupt them each iter; re-zero)
                zp5 = st["zp5"]
                for k in range(CT):
                    for b in range(BP):
                        nc.gpsimd.memset(zp5[:, k, b, 0, :], 0.0)
                        nc.gpsimd.memset(zp5[:, k, b, HP - 1, :], 0.0)
                        nc.gpsimd.memset(zp5[:, k, b, :, 0], 0.0)
                        nc.gpsimd.memset(zp5[:, k, b, :, WP - 1], 0.0)

            def phase_a1(st, b):
                vecs, zp5, outa = st["vecs"], st["zp5"], st["outa"]
                for s in range(NSC):
                    for ct in range(CT):
                        xt = stream.tile([128, SR * W], F32, tag="xs", bufs=4)
                        nc.sync.dma_start(
                            xt[:], x_v[ct][:, b, s * SR * W:(s + 1) * SR * W])
                        zdst = zp5[:, ct, b, 1 + s * SR:1 + (s + 1) * SR, 1:1 + W]
                        nc.scalar.activation(
                            zdst, xt[:].rearrange("p (r w) -> p r w", w=W),
                            AF.Sign, bias=vecs[ct][:, V_MOVE:V_MOVE + 1], scale=1.0)
                        seg = slice(b * HW + s * SR * W, b * HW + (s + 1) * SR * W)
                        # prelu(x, a1) = max(a1*x, x) since 0 <= a1 < 1
                        nc.vector.scalar_tensor_tensor(
                            outa[ct][:, seg], xt[:],
                            vecs[ct][:, V_AL1:V_AL1 + 1], xt[:],
                            op0=ALU.mult, op1=ALU.max)
                        col = b * NSC + s
                        nc.vector.tensor_reduce(
                            st["ps_x"][ct][:, col:col + 1], xt[:],
                            axis=mybir.AxisListType.XY, op=ALU.add)

            def phase_conv(st, b, p0=0, p1=NPAIR):
                vecs, zp5 = st["vecs"], st["zp5"]
                for pi in range(p0, p1):
                    pr = pairs[pi]
                    for mt in range(CT):
                        pt = psum.tile([128, 2, 512], F32, tag="mm")
                        for jj, j in enumerate(pr):
                            for dy in range(3):
                                for dx in range(3):
                                    rhs = zp5[:, :, b,
                                              j * RS + dy:j * RS + dy + RS,
                                              dx:dx + W]
                                    nc.tensor.matmul(
                                        pt[:, jj, 0:RS * W],
                                        st["wconv"][dy * 3 + dx][
                                            :, :, mt * 128:(mt + 1) * 128],
                                        rhs,
                                        start=(dy == 0 and dx == 0),
                                        stop=(dy == 2 and dx == 2),
                                        perf_mode=PM.DoubleRow)
                        col = b * NPAIR + pi
                        j0 = pr[0]
                        n = len(pr)
                        ydst = st["y1"][mt][b][:, j0 * RS * W:(j0 + n) * RS * W]
                        if n == 2:
                            ydst = ydst.rearrange("p (n f) -> p n f", n=2)
                            src = pt[:, 0:2, 0:RS * W]
                        else:
                            src = pt[:, 0, 0:RS * W]
                        nc.scalar.activation(
                            ydst, src, AF.Prelu,
                            bias=vecs[mt][:, V_BIAS1:V_BIAS1 + 1],
                            scale=vecs[mt][:, V_INV1:V_INV1 + 1],
                            alpha=vecs[mt][:, V_AL2:V_AL2 + 1],
                            accum_out=st["ps_y1"][mt][:, col:col + 1])

            def se_gate(st, b, ps1, ps2, n1, n2, w1k, w2, b1t, vb2, vpost, gcol,
                        sum_corr_col=None):
                """SE gate for image b: gate[gcol+b] = post * sigmoid(...)"""
                vecs, sxr, sums2 = st["vecs"], st["sxr"], st["sums2"]
                gtag = f"g{st['uid']}_{gcol}_{b}"
                for ct in range(CT):
                    nc.vector.tensor_reduce(
                        sxr[ct][:, 0:1],
                        ps1[ct][:, b * n1:(b + 1) * n1],
                        axis=mybir.AxisListType.X, op=ALU.add)
                    if sum_corr_col is not None:
                        nc.vector.tensor_scalar(
                            sxr[ct][:, 0:1], sxr[ct][:, 0:1],
                            vecs[ct][:, sum_corr_col:sum_corr_col + 1], None,
                            op0=ALU.add)
                    nc.vector.tensor_reduce(
                        sxr[ct][:, 1:2],
                        ps2[ct][:, b * n2:(b + 1) * n2],
                        axis=mybir.AxisListType.X, op=ALU.add)
                hp = psum_se.tile([32, 1], F32, tag="seh")
                first = True
                for k in range(2):
                    for ct in range(CT):
                        nc.tensor.matmul(hp[:], w1k[k][ct][:],
                                         sxr[ct][:, k:k + 1],
                                         start=first,
                                         stop=(k == 1 and ct == CT - 1))
                        first = False
                hs = consts.tile([32, 1], F32, tag="hs_" + gtag)
                nc.scalar.activation(hs[:], hp[:], AF.Relu, bias=b1t[:], scale=1.0)
                for mt in range(CT):
                    gp = psum_se.tile([128, 1], F32, tag="seg")
                    nc.tensor.matmul(gp[:], w2[:, mt * 128:(mt + 1) * 128], hs[:],
                                     start=True, stop=True)
                    # inv2/finv2 are folded into the bn1 drain scales on the
                    # host, so the gate is the raw sigmoid
                    nc.scalar.activation(
                        sums2[mt][:, gcol + b:gcol + b + 1], gp[:], AF.Sigmoid,
                        bias=vecs[mt][:, vb2:vb2 + 1], scale=1.0)

            def phase_a4(st, b):
                vecs, outa, zpad = st["vecs"], st["outa"], st["zpad"]
                for ct in range(CT):
                    for s in range(NSC2):
                        seg = slice(b * HW + s * SR2 * W,
                                    b * HW + (s + 1) * SR2 * W)
                        yseg = slice(s * SR2 * W, (s + 1) * SR2 * W)
                        t = stream.tile([128, SR2 * W], BF16, tag="work", bufs=4)
                        nc.vector.tensor_scalar(
                            t[:], st["y1"][ct][b][:, yseg],
                            st["sums2"][ct][:, 8 + b:9 + b], 1.0,
                            op0=ALU.mult, op1=ALU.add)
                        col = b * NSC2 + s
                        nc.vector.scalar_tensor_tensor(
                            outa[ct][:, seg], t[:], 0.0, outa[ct][:, seg],
                            op0=ALU.bypass, op1=ALU.mult,
                            accum_out=st["ps_oa"][ct][:, col:col + 1])
                        z2dst = zpad[:, ct, b * HW + s * SR2 * W:
                                     b * HW + (s + 1) * SR2 * W]
                        nc.scalar.activation(
                            z2dst, outa[ct][:, seg], AF.Sign,
                            bias=vecs[ct][:, V_ZB2:V_ZB2 + 1], scale=1.0)

            def phase_f1(st, b):
                vecs, zpad = st["vecs"], st["zpad"]
                for j in range(NCH):
                    seg = slice(b * HW + j * RS * W, b * HW + (j + 1) * RS * W)
                    for mt in range(CT):
                        pt = psum.tile([128, 512], F32, tag="mmf")
                        nc.tensor.matmul(
                            pt[:, 0:RS * W],
                            st["wffn"][:, :, mt * 128:(mt + 1) * 128],
                            zpad[:, :, seg], start=True, stop=True,
                            perf_mode=PM.DoubleRow)
                        col = b * NCH + j
                        nc.scalar.activation(
                            st["y1"][mt][b][:, j * RS * W:(j + 1) * RS * W],
                            pt[:, 0:RS * W], AF.Prelu,
                            bias=vecs[mt][:, V_FBIAS1:V_FBIAS1 + 1],
                            scale=vecs[mt][:, V_FINV1:V_FINV1 + 1],
                            alpha=vecs[mt][:, V_FAL2:V_FAL2 + 1],
                            accum_out=st["ps_u"][mt][:, col:col + 1])

            def phase_f3(st, b):
                vecs, outa = st["vecs"], st["outa"]
                for ct in range(CT):
                    for s in range(NSC2):
                        seg = slice(b * HW + s * SR2 * W,
                                    b * HW + (s + 1) * SR2 * W)
                        yseg = slice(s * SR2 * W, (s + 1) * SR2 * W)
                        t = stream.tile([128, SR2 * W], BF16, tag="work", bufs=4)
                        nc.vector.tensor_scalar(
                            t[:], st["y1"][ct][b][:, yseg],
                            st["sums2"][ct][:, 10 + b:11 + b],
                            vecs[ct][:, V_CFIN:V_CFIN + 1],
                            op0=ALU.mult, op1=ALU.add)
                        fin = stream.tile([128, SR2 * W], BF16, tag="fin", bufs=4)
                        nc.vector.tensor_tensor(
                            fin[:], t[:], outa[ct][:, seg], op=ALU.add)
                        nc.gpsimd.dma_start(
                            out_v[ct][:, b, s * SR2 * W:(s + 1) * SR2 * W],
                            fin[:])

            def body(st):
                load_consts(st)
                if phase_limit >= 1:
                    phase_a1(st, 0)
                    phase_a1(st, 1)
                if phase_limit >= 2:
                    phase_conv(st, 0)
                    phase_conv(st, 1, 0, 2)
                if phase_limit >= 3:
                    se_gate(st, 0, st["ps_x"], st["ps_y1"], NSC, NPAIR,
                            st["w1a"], st["w2a"], st["b1a"], V_B2A, V_INV2, 8)
                if phase_limit >= 4:
                    phase_a4(st, 0)
                if phase_limit >= 2:
                    phase_conv(st, 1, 2, NPAIR)
                if phase_limit >= 5:
                    phase_f1(st, 0)
                if phase_limit >= 3:
                    se_gate(st, 1, st["ps_x"], st["ps_y1"], NSC, NPAIR,
                            st["w1a"], st["w2a"], st["b1a"], V_B2A, V_INV2, 8)
                if phase_limit >= 6:
                    se_gate(st, 0, st["ps_oa"], st["ps_u"], NSC2, NCH,
                            st["w1f"], st["w2f"], st["b1f"], V_B2F, V_FINV2, 10,
                            sum_corr_col=V_SB2HW)
                if phase_limit >= 4:
                    phase_a4(st, 1)
                if phase_limit >= 7:
                    phase_f3(st, 0)
                if phase_limit >= 5:
                    phase_f1(st, 1)
                if phase_limit >= 6:
                    se_gate(st, 1, st["ps_oa"], st["ps_u"], NSC2, NCH,
                            st["w1f"], st["w2f"], st["b1f"], V_B2F, V_FINV2, 10,
                            sum_corr_col=V_SB2HW)
                if phase_limit >= 7:
                    phase_f3(st, 1)

            for st in states:
                body(st)

    nc.compile()
    return nc


def _host_prep(inputs):
    import ml_dtypes
    f32 = np.float32
    f8 = ml_dtypes.float8_e4m3
    g1, be1, m1, v1 = (inputs["a_bn1"][i].astype(f32) for i in range(4))
    g2, be2, m2, v2 = (inputs["a_bn2"][i].astype(f32) for i in range(4))
    fg1, fbe1, fm1, fv1 = (inputs["f_bn1"][i].astype(f32) for i in range(4))
    fg2, fbe2, fm2, fv2 = (inputs["f_bn2"][i].astype(f32) for i in range(4))
    inv1 = g1 / np.sqrt(v1 + EPS)
    bias1 = be1 - m1 * inv1
    inv2 = g2 / np.sqrt(v2 + EPS)
    bias2 = be2 - m2 * inv2
    finv1 = fg1 / np.sqrt(fv1 + EPS)
    fbias1 = fbe1 - fm1 * finv1
    finv2 = fg2 / np.sqrt(fv2 + EPS)
    fbias2 = fbe2 - fm2 * finv2

    s = inputs["a_scale"].astype(f32)
    fs = inputs["f_scale"].astype(f32)

    # inv2/finv2 are strictly positive for this model family (gamma ~ 1+-0.1),
    # so they commute with prelu and fold into the bn1 drain affine
    assert np.all(inv2 > 0) and np.all(finv2 > 0)
    vecs = np.zeros((C, NV), f32)
    vecs[:, 0] = inputs["a_move"]
    vecs[:, 1] = inputs["a_alpha1"]
    vecs[:, 2] = inv1 * inv2 / WSCL
    vecs[:, 3] = bias1 * inv2
    vecs[:, 4] = inputs["a_alpha2"]
    vecs[:, 5] = s
    vecs[:, 6] = 1.0 - s
    vecs[:, 7] = inv2
    vecs[:, 8] = bias2 + inputs["f_move"]
    vecs[:, 9] = finv1 * finv2 / WSCL
    vecs[:, 10] = fbias1 * finv2
    vecs[:, 11] = inputs["f_alpha2"]
    vecs[:, 12] = fs
    vecs[:, 13] = 1.0 - fs
    vecs[:, 14] = finv2
    vecs[:, 15] = fbias2 + bias2
    vecs[:, 16] = inputs["a_se_b2"]
    vecs[:, 17] = inputs["f_se_b2"]
    vecs[:, 18] = float(HW) * bias2
    vecs[:, 19] = bias2
    vecs_ct = np.ascontiguousarray(vecs.reshape(CT, 128, NV))

    bw = np.clip(inputs["a_w"].astype(f32), -1.0, 1.0) * WSCL
    bwT = np.transpose(bw, (1, 0, 2, 3))     # [cin, cout, ky, kx]
    wa_h = np.zeros((9, 128, CT, C), f8)
    for ky in range(3):
        for kx in range(3):
            m = bwT[:, :, ky, kx].reshape(CT, 128, C)   # [k, p, cout]
            wa_h[ky * 3 + kx] = m.transpose(1, 0, 2).astype(f8)
    bw2 = np.clip(inputs["f_w"].astype(f32), -1.0, 1.0) * WSCL
    wfm = bw2.T.reshape(CT, 128, C).transpose(1, 0, 2).astype(f8)

    def w1_fold(w1, sv, dv):
        # pooled y/u sums arrive pre-scaled by inv2/finv2 (drain fold): undo
        w1t = w1.astype(f32).T / float(HW)          # [256, 32]
        out = np.zeros((2, CT, 128, 32), f32)
        out[0] = (w1t * sv[:, None]).reshape(CT, 128, 32)
        out[1] = (w1t * ((1.0 - sv) / dv)[:, None]).reshape(CT, 128, 32)
        return out

    common = {
        "wa": np.ascontiguousarray(wa_h), "wf": np.ascontiguousarray(wfm),
        "vecs": vecs_ct,
        "b1a": inputs["a_se_b1"].astype(f32).reshape(32, 1),
        "b1f": inputs["f_se_b1"].astype(f32).reshape(32, 1),
        "w1a": w1_fold(inputs["a_se_w1"], s, inv2),
        "w2a": np.ascontiguousarray(inputs["a_se_w2"].astype(f32).T),
        "w1f": w1_fold(inputs["f_se_w1"], fs, finv2),
        "w2f": np.ascontiguousarray(inputs["f_se_w2"].astype(f32).T),
    }
    return common


def kernel(**inputs):
    from concourse import bass_utils

    if "nc" not in _CACHE:
        _CACHE["nc"] = _build_program()
    nc = _CACHE["nc"]

    common = _host_prep(inputs)
    x = np.ascontiguousarray(inputs["x"].astype(np.float32))
    in_maps = []
    for c in range(NCORES):
        m = dict(common)
        m["xin"] = np.ascontiguousarray(x[c * BP:(c + 1) * BP])
        in_maps.append(m)

    res = None
    for attempt in range(3):
        try:
            res = bass_utils.run_bass_kernel_spmd(
                nc, in_maps, core_ids=list(range(NCORES)))
            break
        except Exception:
            # transient device wedge on a freshly loaded NEFF: retry
            if attempt == 2:
                raise
    out = np.empty((B, C, H, W), np.float32)
    for c in range(NCORES):
        out[c * BP:(c + 1) * BP] = res.results[c]["out"]
    return out
